# revision 4
# baseline (speedup 1.0000x reference)
"""AdaConv2d fused kernel for 8 TRN2 NeuronCores (pure data parallel).

Per-sample pipeline (all fused on-chip):
  1. instance-norm stats (mean/var over HW)
  2. dynamic per-(b,c) depthwise 3x3 conv with reflect padding
  3. per-(b,c) scale+bias (folded algebraically into the depthwise taps:
     y = A*(sum_t w_t * x_t) + B with A = rstd*w_pt, B = bias - mu*A*sum(w))
  4. fixed 3x3 conv (256->256) with reflect padding, as 18 accumulated
     bf16 matmuls per 512-pixel PSUM block

Layout: channels on partitions (2 tiles of 128), pixels on the free axis.
Padded image buffers are 66 rows x 68 cols (row stride 68 keeps 4-byte
alignment for DVE 2x bf16 mode; cols 0 and 67 are junk, col 1 / col 66 are
the reflect pads, rows 0 / 65 are the reflect pad rows).
"""

import math
import os
from contextlib import ExitStack

import numpy as np

B_GLOBAL = 32
N_CORES = 8
NB = B_GLOBAL // N_CORES  # batches per core
C = 256
H = W = 64
HP = H + 2          # 66 padded rows
WP = W + 4          # 68 padded cols (stride), data at cols 2..65
NPIX = H * W        # 4096
CT = C // 128       # channel tiles
OT = C // 128       # out-channel tiles
EPS = 1e-5
BLK_ROWS = 8        # output rows per PSUM block
NBLK = H // BLK_ROWS  # 8 blocks of N=512

_CACHED = {}


def _build(nb=NB):
    import concourse.bass as bass
    import concourse.mybir as mybir
    import concourse.tile as tile
    from concourse import bacc

    f32 = mybir.dt.float32
    bf16 = mybir.dt.bfloat16
    AF = mybir.ActivationFunctionType
    ALU = mybir.AluOpType

    nc = bacc.Bacc(None, target_bir_lowering=False)

    x_ext = nc.declare_dram_parameter("x", [nb, C, H, W], f32, isOutput=False)
    wsp_ext = nc.declare_dram_parameter("wsp", [nb, CT, 128, 9], f32, isOutput=False)
    wpt_ext = nc.declare_dram_parameter("wpt", [nb, CT, 128], f32, isOutput=False)
    bis_ext = nc.declare_dram_parameter("bis", [nb, CT, 128], f32, isOutput=False)
    cw_ext = nc.declare_dram_parameter("cw", [CT, 128, 3, 3, OT, 128], bf16, isOutput=False)
    cb_ext = nc.declare_dram_parameter("cb", [OT, 128], f32, isOutput=False)
    out_ext = nc.declare_dram_parameter("out", [nb, C, H, W], f32, isOutput=True)

    with tile.TileContext(nc) as tc, ExitStack() as ctx:
        singles = ctx.enter_context(tc.tile_pool(name="singles", bufs=1))
        xin_pool = ctx.enter_context(tc.tile_pool(name="xin", bufs=2))
        xpb_pool = ctx.enter_context(tc.tile_pool(name="xpb", bufs=2))
        xpb2_pool = ctx.enter_context(tc.tile_pool(name="xpb2", bufs=2))
        yp_pool = ctx.enter_context(tc.tile_pool(name="yp", bufs=4))
        stage_pool = ctx.enter_context(tc.tile_pool(name="stage", bufs=2))
        small_pool = ctx.enter_context(tc.tile_pool(name="small", bufs=4))
        psum_pool = ctx.enter_context(tc.tile_pool(name="psum", bufs=8, space="PSUM"))

        # ---- constants / fixed weights ----
        cw_sb = []
        for ct in range(CT):
            t = singles.tile([128, 3, 3, OT, 128], bf16, tag=f"cw{ct}")
            nc.sync.dma_start(out=t[:], in_=cw_ext[ct])
            cw_sb.append(t)
        cb_sb = singles.tile([128, OT], f32, tag="cb")
        for ot in range(OT):
            nc.sync.dma_start(out=cb_sb[:, ot : ot + 1], in_=cb_ext[ot, :, None])
        eps_sb = singles.tile([128, 1], f32, tag="eps")
        nc.vector.memset(eps_sb[:], EPS)

        yp_tiles = {}

        def produce_yp(b, ct):
            """depthwise+norm pipeline for one (batch, channel-tile)."""
            xf = xin_pool.tile([128, H, W], f32, tag="xf")
            nc.sync.dma_start(out=xf[:], in_=x_ext[b, ct * 128 : (ct + 1) * 128])

            wsp = small_pool.tile([128, 9], f32, tag="wsp")
            nc.sync.dma_start(out=wsp[:], in_=wsp_ext[b, ct])
            wpt = small_pool.tile([128, 1], f32, tag="wpt")
            nc.sync.dma_start(out=wpt[:], in_=wpt_ext[b, ct, :, None])
            bis = small_pool.tile([128, 1], f32, tag="bis")
            nc.sync.dma_start(out=bis[:], in_=bis_ext[b, ct, :, None])

            xpb = xpb_pool.tile([128, HP, WP], bf16, tag="xpb")
            xpb2 = xpb2_pool.tile([128, HP, WP], bf16, tag="xpb2")

            # stats: sum(x) fused into the f32->bf16 convert; sum(x^2) via
            # ACT Square writing into xpb2's buffer (overwritten later).
            sumx = small_pool.tile([128, 1], f32, tag="sumx")
            sumsq = small_pool.tile([128, 1], f32, tag="sumsq")
            sq_scratch = xpb2[:].rearrange("p a b -> p (a b)")[:, :NPIX]
            nc.scalar.activation(
                out=sq_scratch, in_=xf[:].rearrange("p a b -> p (a b)"),
                func=AF.Square, accum_out=sumsq[:],
            )
            # zero junk columns 0 and 67 (their values feed only junk psum
            # columns, but keep them finite)
            nc.gpsimd.memset(xpb[:, :, 0:2], 0.0)
            nc.gpsimd.memset(xpb[:, :, 66:68], 0.0)
            nc.scalar.activation(
                out=xpb[:, 1 : 1 + H, 2 : 2 + W], in_=xf[:],
                func=AF.Copy, accum_out=sumx[:],
            )
            # reflect pads: col 1 <- x col 1 (at 3), col 66 <- x col 62 (at 64)
            nc.scalar.copy(out=xpb[:, 1 : 1 + H, 1:2], in_=xpb[:, 1 : 1 + H, 3:4])
            nc.scalar.copy(out=xpb[:, 1 : 1 + H, 66:67], in_=xpb[:, 1 : 1 + H, 64:65])
            # row 0 <- row 2, row 65 <- row 63 (full width, pads included)
            nc.scalar.copy(out=xpb[:, 0], in_=xpb[:, 2])
            nc.scalar.copy(out=xpb[:, HP - 1], in_=xpb[:, HP - 3])

            # shifted copy (one element left) for 4B-aligned odd-column taps
            xpb_flat = xpb[:].rearrange("p a b -> p (a b)")
            xpb2_flat = xpb2[:].rearrange("p a b -> p (a b)")
            ntot = HP * WP
            nc.gpsimd.tensor_copy(out=xpb2_flat[:, 0 : ntot - 1], in_=xpb_flat[:, 1:ntot])
            nc.gpsimd.memset(xpb2_flat[:, ntot - 1 : ntot], 0.0)

            # ---- stats finalize (tiny per-partition ops) ----
            mu = small_pool.tile([128, 1], f32, tag="mu")
            nc.vector.tensor_scalar_mul(mu[:], sumx[:], 1.0 / NPIX)
            m2 = small_pool.tile([128, 1], f32, tag="m2")
            nc.vector.tensor_scalar_mul(m2[:], sumsq[:], 1.0 / NPIX)
            musq = small_pool.tile([128, 1], f32, tag="musq")
            nc.vector.tensor_mul(musq[:], mu[:], mu[:])
            var = small_pool.tile([128, 1], f32, tag="var")
            nc.vector.tensor_sub(var[:], m2[:], musq[:])
            std = small_pool.tile([128, 1], f32, tag="std")
            nc.scalar.activation(out=std[:], in_=var[:], func=AF.Sqrt, bias=eps_sb[:])
            rstd = small_pool.tile([128, 1], f32, tag="rstd")
            nc.vector.reciprocal(out=rstd[:], in_=std[:])
            a_sc = small_pool.tile([128, 1], f32, tag="a_sc")
            nc.vector.tensor_mul(a_sc[:], rstd[:], wpt[:])
            sw = small_pool.tile([128, 1], f32, tag="sw")
            nc.vector.reduce_sum(sw[:], wsp[:], axis=mybir.AxisListType.X)
            t1 = small_pool.tile([128, 1], f32, tag="t1")
            nc.vector.tensor_mul(t1[:], mu[:], a_sc[:])
            nc.vector.tensor_mul(t1[:], t1[:], sw[:])
            bconst = small_pool.tile([128, 1], f32, tag="bconst")
            nc.vector.tensor_sub(bconst[:], bis[:], t1[:])
            # scaled taps (scalar operands must be f32)
            wsc = small_pool.tile([128, 9], f32, tag="wsc")
            nc.vector.tensor_scalar_mul(wsc[:], wsp[:], a_sc[:])

            # ---- depthwise: 9 taps on DVE, norm-affine folded in ----
            yp = yp_pool.tile([128, HP, WP], bf16, tag="yp")
            yp_int = yp[:, 1 : 1 + H, 2 : 2 + W]

            def tap_src(dh, dw):
                if dw == 0:
                    return xpb[:, 1 + dh : 1 + dh + H, 2 : 2 + W]
                return xpb2[:, 1 + dh : 1 + dh + H, 1 + dw : 1 + dw + W]

            first = True
            for dh in (-1, 0, 1):
                for dw in (-1, 0, 1):
                    t = (dh + 1) * 3 + (dw + 1)
                    src = tap_src(dh, dw)
                    if first:
                        nc.vector.tensor_scalar(
                            yp_int, src, wsc[:, t : t + 1], bconst[:],
                            op0=ALU.mult, op1=ALU.add,
                        )
                        first = False
                    else:
                        nc.vector.scalar_tensor_tensor(
                            out=yp_int, in0=src, scalar=wsc[:, t : t + 1],
                            in1=yp_int, op0=ALU.mult, op1=ALU.add,
                        )

            # yp reflect borders
            nc.gpsimd.memset(yp[:, :, 0:2], 0.0)
            nc.gpsimd.memset(yp[:, :, 66:68], 0.0)
            nc.scalar.copy(out=yp[:, 1 : 1 + H, 1:2], in_=yp[:, 1 : 1 + H, 3:4])
            nc.scalar.copy(out=yp[:, 1 : 1 + H, 66:67], in_=yp[:, 1 : 1 + H, 64:65])
            nc.scalar.copy(out=yp[:, 0], in_=yp[:, 2])
            nc.scalar.copy(out=yp[:, HP - 1], in_=yp[:, HP - 3])
            yp_tiles[(b, ct)] = yp

        def big_conv(b):
            for ot in range(OT):
                stage = stage_pool.tile([128, NBLK, BLK_ROWS * W], f32, tag="stage")
                for half in range(2):
                    blks = range(half * NBLK // 2, (half + 1) * NBLK // 2)
                    ps = {}
                    for blk in blks:
                        ps[blk] = psum_pool.tile(
                            [128, BLK_ROWS * W], f32, tag="ps",
                            name=f"ps_{b}_{ot}_{half}_{blk}",
                        )
                    n_acc = CT * 9
                    i = 0
                    for ct in range(CT):
                        yp = yp_tiles[(b, ct)]
                        for dh in (-1, 0, 1):
                            for dw in (-1, 0, 1):
                                kh, kw = dh + 1, dw + 1
                                lhsT = cw_sb[ct][:, kh, kw, ot, :]
                                for blk in blks:
                                    r0 = blk * BLK_ROWS
                                    rhs = yp[:, r0 + 1 + dh : r0 + 1 + dh + BLK_ROWS,
                                             2 + dw : 2 + dw + W]
                                    nc.tensor.matmul(
                                        ps[blk][:], lhsT, rhs,
                                        start=(i == 0), stop=(i == n_acc - 1),
                                    )
                                i += 1
                    for blk in blks:
                        nc.scalar.activation(
                            out=stage[:, blk], in_=ps[blk][:],
                            func=AF.Identity, bias=cb_sb[:, ot : ot + 1],
                        )
                nc.sync.dma_start(
                    out=out_ext[b, ot * 128 : (ot + 1) * 128],
                    in_=stage[:].rearrange("p a b -> p (a b)").rearrange(
                        "p (h w) -> p h w", w=W),
                )

        for b in range(nb):
            for ct in range(CT):
                produce_yp(b, ct)
            big_conv(b)

    nc.compile()
    return nc


def _host_prep(x, w_spatial, w_pointwise, bias, conv_w, conv_b, nb=NB):
    import ml_dtypes

    ncores = x.shape[0] // nb
    cw = np.ascontiguousarray(
        conv_w.reshape(OT, 128, CT, 128, 3, 3).transpose(2, 3, 4, 5, 0, 1)
    ).astype(ml_dtypes.bfloat16)
    cb = np.ascontiguousarray(conv_b.reshape(OT, 128)).astype(np.float32)
    wsp = np.ascontiguousarray(w_spatial.reshape(-1, CT, 128, 9)).astype(np.float32)
    wpt = np.ascontiguousarray(w_pointwise.reshape(-1, CT, 128)).astype(np.float32)
    bis = np.ascontiguousarray(bias.reshape(-1, CT, 128)).astype(np.float32)
    x = np.ascontiguousarray(x).astype(np.float32)
    in_maps = []
    for i in range(ncores):
        sl = slice(i * nb, (i + 1) * nb)
        in_maps.append({
            "x": np.ascontiguousarray(x[sl]),
            "wsp": np.ascontiguousarray(wsp[sl]),
            "wpt": np.ascontiguousarray(wpt[sl]),
            "bis": np.ascontiguousarray(bis[sl]),
            "cw": cw,
            "cb": cb,
        })
    return in_maps


def _run(inputs, trace=False):
    from concourse.bass_utils import run_bass_kernel_spmd

    if "nc" not in _CACHED:
        _CACHED["nc"] = _build()
    nc = _CACHED["nc"]
    in_maps = _host_prep(**inputs)
    res = run_bass_kernel_spmd(
        nc, in_maps, core_ids=list(range(N_CORES)), trace=trace
    )
    out = np.concatenate([res.results[i]["out"] for i in range(N_CORES)], axis=0)
    return out.astype(np.float32), res


def kernel(x, w_spatial, w_pointwise, bias, conv_w, conv_b):
    out, _ = _run(
        dict(x=np.asarray(x), w_spatial=np.asarray(w_spatial),
             w_pointwise=np.asarray(w_pointwise), bias=np.asarray(bias),
             conv_w=np.asarray(conv_w), conv_b=np.asarray(conv_b)),
        trace=bool(int(os.environ.get("KERNEL_TRACE", "0"))),
    )
    return out
